# revision 1
# baseline (speedup 1.0000x reference)
"""Neural CDE (RK4 scan over a tiny MLP vector field) on 8 TRN2 cores.

Strategy: pure batch data-parallelism (1024 batch -> 128 per core). State is
kept transposed hT [H=32, B=128] (batch on the free axis) so every matmul
contracts over the small feature dims on PE partitions. The einsum
g = f(h).dX, the RK4 state updates h + c*k, and the next substep's first
matmul are all fused into PE PSUM-accumulated matmuls:

    z1(s+1) = W1aug^T hTaug (pre-issued base) + c_s (R W1)^T u_s
    h_next  = I^T h + sum_s (w_s/6) R^T u_s          (R = tiled identity)

where u_s = tanh(W4^T z3 + b4) * broadcast(dX). Biases are folded into the
matmuls via an augmented ones-row, so the inter-layer nonlinearities are pure
max(x, 0) on DVE and one tanh on ACT per substep.

The critical path per substep is 9 engine ops:
  red-mms(PE) -> relu1(DVE) -> MM2(PE) -> relu2 -> MM3(PE) -> relu3
  -> MM4A/B(PE) -> tanh(ACT) -> mult(DVE)
"""

import numpy as np
from contextlib import ExitStack

import concourse.bass as bass
import concourse.tile as tile
from concourse import bacc, mybir
from concourse.bass_utils import run_bass_kernel_spmd

B, T, D, H, HH = 1024, 1000, 6, 32, 15
NCORES = 8
P = B // NCORES          # 128 batch per core
TS_FULL = T - 1          # 999 scan steps
CH = 32                  # dx chunk size (steps per DMA)

F32 = mybir.dt.float32
R32 = mybir.dt.float32r
TANH = mybir.ActivationFunctionType.Tanh
USE_F32R = False


def _r(ap):
    """View an fp32 AP as float32r (same bits, single-pass PE matmul mode)."""
    return ap.bitcast(R32) if USE_F32R else ap


def _emit(ctx, tc, ins, out_ap, ts):
    nc = tc.nc
    nchunk = (ts + CH - 1) // CH
    sb = ctx.enter_context(tc.tile_pool(name="sb", bufs=1))
    ps = ctx.enter_context(tc.tile_pool(name="ps", bufs=1, space="PSUM"))

    # ---- persistent SBUF tiles ----
    w1 = sb.tile([H + 1, HH], F32, name="w1")
    w2 = sb.tile([HH + 1, HH], F32, name="w2")
    w3 = sb.tile([HH + 1, HH], F32, name="w3")
    w4a = sb.tile([HH + 1, 96], F32, name="w4a")
    w4b = sb.tile([HH + 1, 96], F32, name="w4b")
    rw1_h = sb.tile([96, HH], F32, name="rw1_h")   # 0.5 * tile(W1)
    rw1_1 = sb.tile([96, HH], F32, name="rw1_1")   # 1.0 *
    rw1_6 = sb.tile([96, HH], F32, name="rw1_6")   # 1/6 *
    rw1_3 = sb.tile([96, HH], F32, name="rw1_3")   # 1/3 *
    rsel_6 = sb.tile([96, H], F32, name="rsel_6")  # 1/6 * tile(I)
    rsel_3 = sb.tile([96, H], F32, name="rsel_3")  # 1/3 *
    eye = sb.tile([H, H], F32, name="eye")
    sel_a = sb.tile([D, 96], F32, name="sel_a")
    sel_b = sb.tile([D, 96], F32, name="sel_b")
    hA = sb.tile([H + 1, P], F32, name="hA")
    hB = sb.tile([H + 1, P], F32, name="hB")
    z1 = sb.tile([HH + 1, P], F32, name="z1")
    z2 = sb.tile([HH + 1, P], F32, name="z2")
    z3 = sb.tile([HH + 1, P], F32, name="z3")
    tt = sb.tile([96, 2 * P], F32, name="tt")
    uu_a = sb.tile([96, 2 * P], F32, name="uu_a")
    uu_b = sb.tile([96, 2 * P], F32, name="uu_b")
    bc = [sb.tile([96, 2 * P], F32, name=f"bc{i}") for i in range(2)]
    chunk = [sb.tile([D, CH * P], F32, name=f"chunk{i}") for i in range(2)]

    # ---- PSUM tiles (8 banks exactly) ----
    pz1 = [ps.tile([HH, P], F32, name=f"pz1_{s}") for s in range(4)]
    pz23 = ps.tile([HH, 2 * P], F32, name="pz23")
    pf = ps.tile([96, 2 * P], F32, name="pf")
    ph = ps.tile([H, P], F32, name="ph")
    pbc = ps.tile([96, 2 * P], F32, name="pbc")

    # ---- one-time loads ----
    for t_sb, name in [
        (w1, "w1"), (w2, "w2"), (w3, "w3"), (w4a, "w4a"), (w4b, "w4b"),
        (rw1_h, "rw1_h"), (rw1_1, "rw1_1"), (rw1_6, "rw1_6"), (rw1_3, "rw1_3"),
        (rsel_6, "rsel_6"), (rsel_3, "rsel_3"), (eye, "eye"),
        (sel_a, "sel_a"), (sel_b, "sel_b"),
    ]:
        nc.sync.dma_start(out=t_sb[:, :], in_=ins[name][:, :])
    nc.sync.dma_start(out=hA[:, :], in_=ins["h0t"][:, :])
    nc.sync.dma_start(out=chunk[0][:, :], in_=ins["dxc"][0, :, :])
    if nchunk > 1:
        nc.sync.dma_start(out=chunk[1][:, :], in_=ins["dxc"][1, :, :])
    nc.sync.dma_start(out=hB[H:H + 1, :], in_=ins["ones"][:, :])
    nc.sync.dma_start(out=z1[HH:HH + 1, :], in_=ins["ones"][:, :])
    nc.sync.dma_start(out=z2[HH:HH + 1, :], in_=ins["ones"][:, :])
    nc.sync.dma_start(out=z3[HH:HH + 1, :], in_=ins["ones"][:, :])

    # bcast tiles for t=0
    nc.tensor.matmul(pbc[:, 0:P], lhsT=_r(sel_a[:, :]), rhs=_r(chunk[0][:, 0:P]),
                     start=True, stop=True)
    nc.tensor.matmul(pbc[:, P:2 * P], lhsT=_r(sel_b[:, :]), rhs=_r(chunk[0][:, 0:P]),
                     start=True, stop=True)
    nc.scalar.copy(bc[0][:, :], pbc[:, :])
    # substep-0 preactivation for t=0 (no red contributions yet)
    nc.tensor.matmul(pz1[0][:, :], lhsT=_r(w1[:, :]), rhs=_r(hA[:, :]),
                     start=True, stop=True)

    h_state = [hA, hB]                       # h_state[t % 2] holds state(t)
    uu2 = [uu_a, uu_b]                       # u of global substep g in uu2[g%2]
    C_SUB = [rw1_h, rw1_h, rw1_1]            # scale for h + c*k inputs
    W_RW1 = [rw1_6, rw1_3, rw1_3, rw1_6]     # RK4 combine weights into z1s0'
    W_RSEL = [rsel_6, rsel_3, rsel_3, rsel_6]

    def red_pair(dst, lhs, u, stop):
        nc.tensor.matmul(dst, lhsT=_r(lhs[:, :]), rhs=_r(u[:, 0:P]),
                         start=False, stop=False, skip_group_check=True)
        nc.tensor.matmul(dst, lhsT=_r(lhs[:, :]), rhs=_r(u[:, P:2 * P]),
                         start=False, stop=stop, skip_group_check=True)

    def base_mm(dst, lhs, rhs):
        nc.tensor.matmul(dst, lhsT=_r(lhs), rhs=_r(rhs),
                         start=True, stop=False, skip_group_check=True)

    for t in range(ts):
        last = t == ts - 1
        h_cur = h_state[t % 2]
        bct = bc[t % 2]
        # dx chunk prefetch (chunks 0,1 preloaded before the loop)
        ci = t // CH + 1
        if t % CH == 0 and 2 <= ci < nchunk:
            nc.sync.dma_start(out=chunk[ci % 2][:, :], in_=ins["dxc"][ci, :, :])

        for s in range(4):
            g = t * 4 + s
            u_prev = uu2[(g + 1) % 2]
            u_cur = uu2[g % 2]
            # relu of layer-1 preactivation (bias folded into the matmuls);
            # the reductions producing pz1[s] were emitted at the end of the
            # previous substep, ahead of everything below in the PE queue.
            nc.vector.tensor_scalar_max(z1[0:HH, :], pz1[s][:, :], 0.0)
            nc.tensor.matmul(pz23[:, 0:P], lhsT=_r(w2[:, :]), rhs=_r(z1[:, :]),
                             start=True, stop=True, skip_group_check=True)
            nc.vector.tensor_scalar_max(z2[0:HH, :], pz23[:, 0:P], 0.0)
            if s == 0 and t > 0:
                # materialize h(t); its producer (ph final red pair) sits just
                # ahead in the PE queue, so this lands early in the substep
                nc.vector.tensor_copy(h_cur[0:H, :], ph[:, :])
            nc.tensor.matmul(pz23[:, P:2 * P], lhsT=_r(w3[:, :]), rhs=_r(z2[:, :]),
                             start=True, stop=True, skip_group_check=True)
            nc.vector.tensor_scalar_max(z3[0:HH, :], pz23[:, P:2 * P], 0.0)
            nc.tensor.matmul(pf[:, 0:P], lhsT=_r(w4a[:, :]), rhs=_r(z3[:, :]),
                             start=True, stop=True, skip_group_check=True)
            nc.tensor.matmul(pf[:, P:2 * P], lhsT=_r(w4b[:, :]), rhs=_r(z3[:, :]),
                             start=True, stop=True, skip_group_check=True)
            # ---- PE fill work: executes during the tanh/mult gap ----
            if s == 0:
                base_mm(pz1[1][:, :], w1[:, :], h_cur[:, :])
                base_mm(ph[:, :], eye[:, :], h_cur[0:H, :])
                base_mm(pz1[2][:, :], w1[:, :], h_cur[:, :])
            elif s == 1:
                if not last:
                    base_mm(pz1[0][:, :], w1[:, :], h_cur[:, :])
                    red_pair(pz1[0][:, :], W_RW1[0], u_prev, False)
                red_pair(ph[:, :], W_RSEL[0], u_prev, False)
                base_mm(pz1[3][:, :], w1[:, :], h_cur[:, :])
            elif s == 2:
                if not last:
                    red_pair(pz1[0][:, :], W_RW1[1], u_prev, False)
                red_pair(ph[:, :], W_RSEL[1], u_prev, False)
                if not last:
                    tn = t + 1
                    sl = slice((tn % CH) * P, (tn % CH) * P + P)
                    cn = chunk[(tn // CH) % 2]
                    nc.tensor.matmul(pbc[:, 0:P], lhsT=_r(sel_a[:, :]),
                                     rhs=_r(cn[:, sl]), start=True, stop=True,
                                     skip_group_check=True)
                    nc.tensor.matmul(pbc[:, P:2 * P], lhsT=_r(sel_b[:, :]),
                                     rhs=_r(cn[:, sl]), start=True, stop=True,
                                     skip_group_check=True)
                    nc.scalar.copy(bc[tn % 2][:, :], pbc[:, :])
            elif s == 3:
                if not last:
                    red_pair(pz1[0][:, :], W_RW1[2], u_prev, False)
                red_pair(ph[:, :], W_RSEL[2], u_prev, False)
            # ---- tail ----
            nc.scalar.activation(tt[:, :], pf[:, :], TANH)
            nc.vector.tensor_mul(u_cur[:, :], tt[:, :], bct[:, :])
            # on-path reductions of u_cur feeding the next substep
            if s < 3:
                red_pair(pz1[s + 1][:, :], C_SUB[s], u_cur, True)
            else:
                if not last:
                    red_pair(pz1[0][:, :], W_RW1[3], u_cur, True)
                red_pair(ph[:, :], W_RSEL[3], u_cur, True)

    h_fin = h_state[ts % 2]
    nc.vector.tensor_copy(h_fin[0:H, :], ph[:, :])
    nc.sync.dma_start(out=out_ap[:, :], in_=h_fin[0:H, :])


_CACHE = {}


def _input_specs(ts):
    nchunk = (ts + CH - 1) // CH
    return {
        "w1": (H + 1, HH), "w2": (HH + 1, HH), "w3": (HH + 1, HH),
        "w4a": (HH + 1, 96), "w4b": (HH + 1, 96),
        "rw1_h": (96, HH), "rw1_1": (96, HH), "rw1_6": (96, HH),
        "rw1_3": (96, HH),
        "rsel_6": (96, H), "rsel_3": (96, H), "eye": (H, H),
        "sel_a": (D, 96), "sel_b": (D, 96),
        "h0t": (H + 1, P), "dxc": (nchunk, D, CH * P), "ones": (1, P),
    }


def build(ts=TS_FULL):
    if ts in _CACHE:
        return _CACHE[ts]
    nc = bacc.Bacc("TRN2", target_bir_lowering=False, debug=False,
                   enable_asserts=False, num_devices=NCORES)
    ins = {
        name: nc.dram_tensor(name, list(shape), F32, kind="ExternalInput").ap()
        for name, shape in _input_specs(ts).items()
    }
    out_ap = nc.dram_tensor("ht_out", [H, P], F32, kind="ExternalOutput").ap()
    with tile.TileContext(nc, trace_sim=False) as tc:
        with ExitStack() as ctx:
            _emit(ctx, tc, ins, out_ap, ts)
    nc.compile()
    _CACHE[ts] = nc
    return nc


def host_prep(coeffs, W0, b0, W1, b1, W2, b2, W3, b3, W4, b4, ts=TS_FULL):
    f32 = np.float32
    coeffs = np.ascontiguousarray(coeffs, dtype=f32)
    h0 = coeffs[:, 0, :] @ W0.astype(f32) + b0.astype(f32)      # [B, H]
    dX = coeffs[:, 1:ts + 1, :] - coeffs[:, :ts, :]             # [B, ts, D]

    W1 = W1.astype(f32)
    W4r = W4.astype(f32).reshape(HH, H, D)
    W4P = W4r.transpose(0, 2, 1).reshape(HH, D * H)             # cols d*32+i
    b4P = b4.astype(f32).reshape(H, D).T.reshape(D * H)
    RW1 = np.tile(W1, (3, 1)).astype(f32)                       # [96, HH]
    Rsel = np.tile(np.eye(H, dtype=f32), (3, 1))                # [96, H]
    sel_a = np.zeros((D, 96), f32)
    sel_b = np.zeros((D, 96), f32)
    for d in range(3):
        sel_a[d, 32 * d:32 * d + 32] = 1.0
        sel_b[d + 3, 32 * d:32 * d + 32] = 1.0

    shared = {
        "w1": np.concatenate([W1, b1.astype(f32)[None]], 0),
        "w2": np.concatenate([W2.astype(f32), b2.astype(f32)[None]], 0),
        "w3": np.concatenate([W3.astype(f32), b3.astype(f32)[None]], 0),
        "w4a": np.concatenate([W4P[:, :96], b4P[None, :96]], 0),
        "w4b": np.concatenate([W4P[:, 96:], b4P[None, 96:]], 0),
        "rw1_h": (0.5 * RW1), "rw1_1": RW1,
        "rw1_6": (RW1 / 6.0).astype(f32), "rw1_3": (RW1 / 3.0).astype(f32),
        "rsel_6": (Rsel / 6.0).astype(f32), "rsel_3": (Rsel / 3.0).astype(f32),
        "eye": np.eye(H, dtype=f32),
        "sel_a": sel_a, "sel_b": sel_b, "ones": np.ones((1, P), f32),
    }
    shared = {k: np.ascontiguousarray(v, dtype=f32) for k, v in shared.items()}

    nchunk = (ts + CH - 1) // CH
    in_maps = []
    for c in range(NCORES):
        sl = slice(c * P, (c + 1) * P)
        h0t = np.concatenate([h0[sl].T, np.ones((1, P), f32)], 0)
        dxt = dX[sl].transpose(1, 2, 0)                          # [ts, D, P]
        pad = np.zeros((nchunk * CH, D, P), f32)
        pad[:ts] = dxt
        dxc = pad.reshape(nchunk, CH, D, P).transpose(0, 2, 1, 3).reshape(
            nchunk, D, CH * P)
        m = dict(shared)
        m["h0t"] = np.ascontiguousarray(h0t, f32)
        m["dxc"] = np.ascontiguousarray(dxc, f32)
        in_maps.append(m)
    return in_maps


def run_device(in_maps, ts=TS_FULL, **kw):
    nc = build(ts)
    return run_bass_kernel_spmd(nc, in_maps, list(range(NCORES)), **kw)


def kernel(coeffs, W0, b0, W1, b1, W2, b2, W3, b3, W4, b4, Wf, bf):
    in_maps = host_prep(coeffs, W0, b0, W1, b1, W2, b2, W3, b3, W4, b4)
    res = run_device(in_maps)
    hT = np.stack([res.results[c]["ht_out"] for c in range(NCORES)])  # [8,H,P]
    h_all = hT.transpose(0, 2, 1).reshape(B, H)
    return (h_all @ Wf.astype(np.float32) + bf.astype(np.float32)).astype(
        np.float32)



# revision 4
# speedup vs baseline: 1.0006x; 1.0006x over previous
"""Neural CDE (RK4 scan over a tiny MLP vector field) on 8 TRN2 cores.

Strategy: pure batch data-parallelism (1024 batch -> 128 per core). State is
kept transposed hT [H=32, B=128] (batch on the free axis) so every matmul
contracts over the small feature dims on PE partitions. The einsum
g = f(h).dX, the RK4 state updates h + c*k, and the next substep's first
matmul are all fused into PE PSUM-accumulated matmuls:

    z1(s+1) = W1aug^T hTaug (pre-issued base) + c_s (R W1)^T u_s
    h_next  = I^T h + sum_s (w_s/6) R^T u_s          (R = tiled identity)

where u_s = tanh(W4^T z3 + b4) * broadcast(dX). Biases are folded into the
matmuls via an augmented ones-row, so the inter-layer nonlinearities are pure
max(x, 0) on DVE and one tanh on ACT per substep.

The critical path per substep is 9 engine ops:
  red-mms(PE) -> relu1(DVE) -> MM2(PE) -> relu2 -> MM3(PE) -> relu3
  -> MM4A/B(PE) -> tanh(ACT) -> mult(DVE)
"""

import numpy as np
from contextlib import ExitStack

import concourse.bass as bass
import concourse.tile as tile
from concourse import bacc, mybir
from concourse.bass_utils import run_bass_kernel_spmd

B, T, D, H, HH = 1024, 1000, 6, 32, 15
NCORES = 8
P = B // NCORES          # 128 batch per core
TS_FULL = T - 1          # 999 scan steps
CH = 32                  # dx chunk size (steps per DMA)

F32 = mybir.dt.float32
R32 = mybir.dt.float32r
TANH = mybir.ActivationFunctionType.Tanh
USE_F32R = False


def _r(ap):
    """View an fp32 AP as float32r (same bits, single-pass PE matmul mode)."""
    return ap.bitcast(R32) if USE_F32R else ap


def _emit(ctx, tc, ins, out_ap, ts):
    nc = tc.nc
    nchunk = (ts + CH - 1) // CH
    sb = ctx.enter_context(tc.tile_pool(name="sb", bufs=1))
    ps = ctx.enter_context(tc.tile_pool(name="ps", bufs=1, space="PSUM"))

    # ---- persistent SBUF tiles ----
    w1 = sb.tile([H + 1, HH], F32, name="w1")
    w2 = sb.tile([HH + 1, HH], F32, name="w2")
    w3 = sb.tile([HH + 1, HH], F32, name="w3")
    w4a = sb.tile([HH + 1, 96], F32, name="w4a")
    w4b = sb.tile([HH + 1, 96], F32, name="w4b")
    rw1_h = sb.tile([96, HH], F32, name="rw1_h")   # 0.5 * tile(W1)
    rw1_1 = sb.tile([96, HH], F32, name="rw1_1")   # 1.0 *
    rw1_6 = sb.tile([96, HH], F32, name="rw1_6")   # 1/6 *
    rw1_3 = sb.tile([96, HH], F32, name="rw1_3")   # 1/3 *
    rsel_6 = sb.tile([96, H], F32, name="rsel_6")  # 1/6 * tile(I)
    rsel_3 = sb.tile([96, H], F32, name="rsel_3")  # 1/3 *
    eye = sb.tile([H, H], F32, name="eye")
    sel_a = sb.tile([D, 96], F32, name="sel_a")
    sel_b = sb.tile([D, 96], F32, name="sel_b")
    hA = sb.tile([H + 1, P], F32, name="hA")
    hB = sb.tile([H + 1, P], F32, name="hB")
    z1 = sb.tile([HH + 1, P], F32, name="z1")
    z2 = sb.tile([HH + 1, P], F32, name="z2")
    z3 = sb.tile([HH + 1, P], F32, name="z3")
    tt = sb.tile([96, 2 * P], F32, name="tt")
    uu_a = sb.tile([96, 2 * P], F32, name="uu_a")
    uu_b = sb.tile([96, 2 * P], F32, name="uu_b")
    bc = [sb.tile([96, 2 * P], F32, name=f"bc{i}") for i in range(2)]
    chunk = [sb.tile([D, CH * P], F32, name=f"chunk{i}") for i in range(2)]

    # ---- PSUM tiles (8 banks exactly) ----
    pz1 = [ps.tile([HH, P], F32, name=f"pz1_{s}") for s in range(4)]
    pz23 = ps.tile([HH, 2 * P], F32, name="pz23")
    pf = ps.tile([96, 2 * P], F32, name="pf")
    ph = ps.tile([H, P], F32, name="ph")
    pbc = ps.tile([96, 2 * P], F32, name="pbc")

    # ---- one-time loads ----
    for t_sb, name in [
        (w1, "w1"), (w2, "w2"), (w3, "w3"), (w4a, "w4a"), (w4b, "w4b"),
        (rw1_h, "rw1_h"), (rw1_1, "rw1_1"), (rw1_6, "rw1_6"), (rw1_3, "rw1_3"),
        (rsel_6, "rsel_6"), (rsel_3, "rsel_3"), (eye, "eye"),
        (sel_a, "sel_a"), (sel_b, "sel_b"),
    ]:
        nc.sync.dma_start(out=t_sb[:, :], in_=ins[name][:, :])
    nc.sync.dma_start(out=hA[:, :], in_=ins["h0t"][:, :])
    nc.sync.dma_start(out=chunk[0][:, :], in_=ins["dxc"][0, :, :])
    if nchunk > 1:
        nc.sync.dma_start(out=chunk[1][:, :], in_=ins["dxc"][1, :, :])
    nc.sync.dma_start(out=hB[H:H + 1, :], in_=ins["ones"][:, :])
    nc.sync.dma_start(out=z1[HH:HH + 1, :], in_=ins["ones"][:, :])
    nc.sync.dma_start(out=z2[HH:HH + 1, :], in_=ins["ones"][:, :])
    nc.sync.dma_start(out=z3[HH:HH + 1, :], in_=ins["ones"][:, :])

    # bcast tiles for t=0
    nc.tensor.matmul(pbc[:, 0:P], lhsT=_r(sel_a[:, :]), rhs=_r(chunk[0][:, 0:P]),
                     start=True, stop=True)
    nc.tensor.matmul(pbc[:, P:2 * P], lhsT=_r(sel_b[:, :]), rhs=_r(chunk[0][:, 0:P]),
                     start=True, stop=True)
    nc.vector.tensor_copy(bc[0][:, :], pbc[:, :])
    # substep-0 preactivation for t=0 (no red contributions yet)
    nc.tensor.matmul(pz1[0][:, :], lhsT=_r(w1[:, :]), rhs=_r(hA[:, :]),
                     start=True, stop=True)

    h_state = [hA, hB]                       # h_state[t % 2] holds state(t)
    uu2 = [uu_a, uu_b]                       # u of global substep g in uu2[g%2]
    C_SUB = [rw1_h, rw1_h, rw1_1]            # scale for h + c*k inputs
    W_RW1 = [rw1_6, rw1_3, rw1_3, rw1_6]     # RK4 combine weights into z1s0'
    W_RSEL = [rsel_6, rsel_3, rsel_3, rsel_6]

    def red_pair(dst, lhs, u, stop):
        nc.tensor.matmul(dst, lhsT=_r(lhs[:, :]), rhs=_r(u[:, 0:P]),
                         start=False, stop=False, skip_group_check=True)
        nc.tensor.matmul(dst, lhsT=_r(lhs[:, :]), rhs=_r(u[:, P:2 * P]),
                         start=False, stop=stop, skip_group_check=True)

    def base_mm(dst, lhs, rhs):
        nc.tensor.matmul(dst, lhsT=_r(lhs), rhs=_r(rhs),
                         start=True, stop=False, skip_group_check=True)

    for t in range(ts):
        last = t == ts - 1
        h_cur = h_state[t % 2]
        bct = bc[t % 2]
        # dx chunk prefetch (chunks 0,1 preloaded before the loop)
        ci = t // CH + 1
        if t % CH == 0 and 2 <= ci < nchunk:
            nc.sync.dma_start(out=chunk[ci % 2][:, :], in_=ins["dxc"][ci, :, :])

        for s in range(4):
            g = t * 4 + s
            u_prev = uu2[(g + 1) % 2]
            u_cur = uu2[g % 2]
            # relu of layer-1 preactivation (bias folded into the matmuls);
            # the reductions producing pz1[s] were emitted at the end of the
            # previous substep, ahead of everything below in the PE queue.
            nc.vector.tensor_scalar_max(z1[0:HH, :], pz1[s][:, :], 0.0)
            nc.tensor.matmul(pz23[:, 0:P], lhsT=_r(w2[:, :]), rhs=_r(z1[:, :]),
                             start=True, stop=True, skip_group_check=True)
            nc.vector.tensor_scalar_max(z2[0:HH, :], pz23[:, 0:P], 0.0)
            if s == 0 and t > 0:
                # materialize h(t); its producer (ph final red pair) sits just
                # ahead in the PE queue, so this lands early in the substep
                nc.vector.tensor_copy(h_cur[0:H, :], ph[:, :])
            nc.tensor.matmul(pz23[:, P:2 * P], lhsT=_r(w3[:, :]), rhs=_r(z2[:, :]),
                             start=True, stop=True, skip_group_check=True)
            nc.vector.tensor_scalar_max(z3[0:HH, :], pz23[:, P:2 * P], 0.0)
            nc.tensor.matmul(pf[:, 0:P], lhsT=_r(w4a[:, :]), rhs=_r(z3[:, :]),
                             start=True, stop=True, skip_group_check=True)
            nc.tensor.matmul(pf[:, P:2 * P], lhsT=_r(w4b[:, :]), rhs=_r(z3[:, :]),
                             start=True, stop=True, skip_group_check=True)
            # ---- PE fill work: executes during the tanh/mult gap ----
            if s == 0:
                base_mm(pz1[1][:, :], w1[:, :], h_cur[:, :])
                base_mm(ph[:, :], eye[:, :], h_cur[0:H, :])
                base_mm(pz1[2][:, :], w1[:, :], h_cur[:, :])
            elif s == 1:
                if not last:
                    base_mm(pz1[0][:, :], w1[:, :], h_cur[:, :])
                    red_pair(pz1[0][:, :], W_RW1[0], u_prev, False)
                red_pair(ph[:, :], W_RSEL[0], u_prev, False)
                base_mm(pz1[3][:, :], w1[:, :], h_cur[:, :])
            elif s == 2:
                if not last:
                    red_pair(pz1[0][:, :], W_RW1[1], u_prev, False)
                red_pair(ph[:, :], W_RSEL[1], u_prev, False)
                if not last:
                    tn = t + 1
                    sl = slice((tn % CH) * P, (tn % CH) * P + P)
                    cn = chunk[(tn // CH) % 2]
                    nc.tensor.matmul(pbc[:, 0:P], lhsT=_r(sel_a[:, :]),
                                     rhs=_r(cn[:, sl]), start=True, stop=True,
                                     skip_group_check=True)
                    nc.tensor.matmul(pbc[:, P:2 * P], lhsT=_r(sel_b[:, :]),
                                     rhs=_r(cn[:, sl]), start=True, stop=True,
                                     skip_group_check=True)
                    nc.vector.tensor_copy(bc[tn % 2][:, :], pbc[:, :])
            elif s == 3:
                if not last:
                    red_pair(pz1[0][:, :], W_RW1[2], u_prev, False)
                red_pair(ph[:, :], W_RSEL[2], u_prev, False)
            # ---- tail ----
            nc.scalar.activation(tt[:, :], pf[:, :], TANH)
            nc.vector.tensor_mul(u_cur[:, :], tt[:, :], bct[:, :])
            # on-path reductions of u_cur feeding the next substep
            if s < 3:
                red_pair(pz1[s + 1][:, :], C_SUB[s], u_cur, True)
            else:
                if not last:
                    red_pair(pz1[0][:, :], W_RW1[3], u_cur, True)
                red_pair(ph[:, :], W_RSEL[3], u_cur, True)

    h_fin = h_state[ts % 2]
    nc.vector.tensor_copy(h_fin[0:H, :], ph[:, :])
    nc.sync.dma_start(out=out_ap[:, :], in_=h_fin[0:H, :])


_CACHE = {}


def _input_specs(ts):
    nchunk = (ts + CH - 1) // CH
    return {
        "w1": (H + 1, HH), "w2": (HH + 1, HH), "w3": (HH + 1, HH),
        "w4a": (HH + 1, 96), "w4b": (HH + 1, 96),
        "rw1_h": (96, HH), "rw1_1": (96, HH), "rw1_6": (96, HH),
        "rw1_3": (96, HH),
        "rsel_6": (96, H), "rsel_3": (96, H), "eye": (H, H),
        "sel_a": (D, 96), "sel_b": (D, 96),
        "h0t": (H + 1, P), "dxc": (nchunk, D, CH * P), "ones": (1, P),
    }


def build(ts=TS_FULL):
    if ts in _CACHE:
        return _CACHE[ts]
    nc = bacc.Bacc("TRN2", target_bir_lowering=False, debug=False,
                   enable_asserts=False, num_devices=NCORES)
    ins = {
        name: nc.dram_tensor(name, list(shape), F32, kind="ExternalInput").ap()
        for name, shape in _input_specs(ts).items()
    }
    out_ap = nc.dram_tensor("ht_out", [H, P], F32, kind="ExternalOutput").ap()
    with tile.TileContext(nc, trace_sim=False) as tc:
        with ExitStack() as ctx:
            _emit(ctx, tc, ins, out_ap, ts)
    nc.compile()
    _CACHE[ts] = nc
    return nc


def host_prep(coeffs, W0, b0, W1, b1, W2, b2, W3, b3, W4, b4, ts=TS_FULL):
    f32 = np.float32
    coeffs = np.ascontiguousarray(coeffs, dtype=f32)
    h0 = coeffs[:, 0, :] @ W0.astype(f32) + b0.astype(f32)      # [B, H]
    dX = coeffs[:, 1:ts + 1, :] - coeffs[:, :ts, :]             # [B, ts, D]

    W1 = W1.astype(f32)
    W4r = W4.astype(f32).reshape(HH, H, D)
    W4P = W4r.transpose(0, 2, 1).reshape(HH, D * H)             # cols d*32+i
    b4P = b4.astype(f32).reshape(H, D).T.reshape(D * H)
    RW1 = np.tile(W1, (3, 1)).astype(f32)                       # [96, HH]
    Rsel = np.tile(np.eye(H, dtype=f32), (3, 1))                # [96, H]
    sel_a = np.zeros((D, 96), f32)
    sel_b = np.zeros((D, 96), f32)
    for d in range(3):
        sel_a[d, 32 * d:32 * d + 32] = 1.0
        sel_b[d + 3, 32 * d:32 * d + 32] = 1.0

    shared = {
        "w1": np.concatenate([W1, b1.astype(f32)[None]], 0),
        "w2": np.concatenate([W2.astype(f32), b2.astype(f32)[None]], 0),
        "w3": np.concatenate([W3.astype(f32), b3.astype(f32)[None]], 0),
        "w4a": np.concatenate([W4P[:, :96], b4P[None, :96]], 0),
        "w4b": np.concatenate([W4P[:, 96:], b4P[None, 96:]], 0),
        "rw1_h": (0.5 * RW1), "rw1_1": RW1,
        "rw1_6": (RW1 / 6.0).astype(f32), "rw1_3": (RW1 / 3.0).astype(f32),
        "rsel_6": (Rsel / 6.0).astype(f32), "rsel_3": (Rsel / 3.0).astype(f32),
        "eye": np.eye(H, dtype=f32),
        "sel_a": sel_a, "sel_b": sel_b, "ones": np.ones((1, P), f32),
    }
    shared = {k: np.ascontiguousarray(v, dtype=f32) for k, v in shared.items()}

    nchunk = (ts + CH - 1) // CH
    in_maps = []
    for c in range(NCORES):
        sl = slice(c * P, (c + 1) * P)
        h0t = np.concatenate([h0[sl].T, np.ones((1, P), f32)], 0)
        dxt = dX[sl].transpose(1, 2, 0)                          # [ts, D, P]
        pad = np.zeros((nchunk * CH, D, P), f32)
        pad[:ts] = dxt
        dxc = pad.reshape(nchunk, CH, D, P).transpose(0, 2, 1, 3).reshape(
            nchunk, D, CH * P)
        m = dict(shared)
        m["h0t"] = np.ascontiguousarray(h0t, f32)
        m["dxc"] = np.ascontiguousarray(dxc, f32)
        in_maps.append(m)
    return in_maps


def run_device(in_maps, ts=TS_FULL, **kw):
    nc = build(ts)
    return run_bass_kernel_spmd(nc, in_maps, list(range(NCORES)), **kw)


def kernel(coeffs, W0, b0, W1, b1, W2, b2, W3, b3, W4, b4, Wf, bf):
    in_maps = host_prep(coeffs, W0, b0, W1, b1, W2, b2, W3, b3, W4, b4)
    res = run_device(in_maps)
    hT = np.stack([res.results[c]["ht_out"] for c in range(NCORES)])  # [8,H,P]
    h_all = hT.transpose(0, 2, 1).reshape(B, H)
    return (h_all @ Wf.astype(np.float32) + bf.astype(np.float32)).astype(
        np.float32)



# revision 10
# speedup vs baseline: 1.8894x; 1.8882x over previous
"""Neural CDE (RK4 scan over a tiny MLP vector field) on 8 TRN2 cores.

Strategy: pure batch data-parallelism (1024 batch -> 128 per core). State is
kept transposed hT [H=32, B=128] (batch on the free axis) so every matmul
contracts over the small feature dims on PE partitions. The einsum
g = f(h).dX, the RK4 state updates h + c*k, and the next substep's first
matmul are all fused into PE PSUM-accumulated matmuls:

    z1(s+1) = W1aug^T hTaug (pre-issued base) + c_s (R W1)^T u_s
    h_next  = I^T h + sum_s (w_s/6) R^T u_s          (R = tiled identity)

where u_s = tanh(W4^T z3 + b4) * broadcast(dX). Biases are folded into the
matmuls via an augmented ones-row, so the inter-layer nonlinearities are pure
max(x, 0) on DVE and one tanh on ACT per substep.

The critical path per substep is 9 engine ops:
  red-mms(PE) -> relu1(DVE) -> MM2(PE) -> relu2 -> MM3(PE) -> relu3
  -> MM4A/B(PE) -> tanh(ACT) -> mult(DVE)
"""

import numpy as np
from contextlib import ExitStack

import concourse.bass as bass
import concourse.tile as tile
from concourse import bacc, mybir
from concourse.bass_utils import run_bass_kernel_spmd

B, T, D, H, HH = 1024, 1000, 6, 32, 15
NCORES = 8
P = B // NCORES          # 128 batch per core
TS_FULL = T - 1          # 999 scan steps
CH = 32                  # dx chunk size (steps per DMA)

F32 = mybir.dt.float32
R32 = mybir.dt.float32r
TANH = mybir.ActivationFunctionType.Tanh
USE_F32R = True
# Matmul-feeding SBUF tiles are declared float32r (single-pass PE mode);
# DRAM stays fp32 and DMA sources are bitcast to f32r at the transfer.
MM_DT = R32 if USE_F32R else F32


def _r(ap):
    """Matmul operand passthrough (tiles already carry the matmul dtype)."""
    return ap


def _rd(ap):
    """Bitcast an fp32 DRAM AP to float32r for DMA into an f32r tile."""
    return ap.bitcast(R32) if USE_F32R else ap


def _emit(ctx, tc, ins, out_ap, ts):
    nc = tc.nc
    nchunk = (ts + CH - 1) // CH
    sb = ctx.enter_context(tc.tile_pool(name="sb", bufs=1))
    ps = ctx.enter_context(tc.tile_pool(name="ps", bufs=1, space="PSUM"))

    # ---- persistent SBUF tiles ----
    w1 = sb.tile([H + 1, HH], MM_DT, name="w1")
    w2 = sb.tile([HH + 1, HH], MM_DT, name="w2")
    w3 = sb.tile([HH + 1, HH], MM_DT, name="w3")
    w4a = sb.tile([HH + 1, 96], MM_DT, name="w4a")
    w4b = sb.tile([HH + 1, 96], MM_DT, name="w4b")
    rw1_h = sb.tile([96, HH], MM_DT, name="rw1_h")   # 0.5 * tile(W1)
    rw1_1 = sb.tile([96, HH], MM_DT, name="rw1_1")   # 1.0 *
    rw1_6 = sb.tile([96, HH], MM_DT, name="rw1_6")   # 1/6 *
    rw1_3 = sb.tile([96, HH], MM_DT, name="rw1_3")   # 1/3 *
    rsel_6 = sb.tile([96, H], MM_DT, name="rsel_6")  # 1/6 * tile(I)
    rsel_3 = sb.tile([96, H], MM_DT, name="rsel_3")  # 1/3 *
    eye = sb.tile([H, H], MM_DT, name="eye")
    sel_a = sb.tile([D, 96], MM_DT, name="sel_a")
    sel_b = sb.tile([D, 96], MM_DT, name="sel_b")
    hA = sb.tile([H + 1, P], MM_DT, name="hA")
    hB = sb.tile([H + 1, P], MM_DT, name="hB")
    z1 = sb.tile([HH + 1, P], MM_DT, name="z1")
    z2 = sb.tile([HH + 1, P], MM_DT, name="z2")
    z3 = sb.tile([HH + 1, P], MM_DT, name="z3")
    tt = sb.tile([96, 2 * P], F32, name="tt")
    uu_a = sb.tile([96, 2 * P], MM_DT, name="uu_a")
    uu_b = sb.tile([96, 2 * P], MM_DT, name="uu_b")
    bc = [sb.tile([96, 2 * P], F32, name=f"bc{i}") for i in range(2)]
    chunk = [sb.tile([D, CH * P], MM_DT, name=f"chunk{i}") for i in range(2)]

    # ---- PSUM tiles (8 banks exactly) ----
    pz1 = [ps.tile([HH, P], F32, name=f"pz1_{s}") for s in range(4)]
    pz23 = ps.tile([HH, 2 * P], F32, name="pz23")
    pf = ps.tile([96, 2 * P], F32, name="pf")
    ph = ps.tile([H, P], F32, name="ph")
    pbc = ps.tile([96, 2 * P], F32, name="pbc")

    # ---- one-time loads ----
    for t_sb, name in [
        (w1, "w1"), (w2, "w2"), (w3, "w3"), (w4a, "w4a"), (w4b, "w4b"),
        (rw1_h, "rw1_h"), (rw1_1, "rw1_1"), (rw1_6, "rw1_6"), (rw1_3, "rw1_3"),
        (rsel_6, "rsel_6"), (rsel_3, "rsel_3"), (eye, "eye"),
        (sel_a, "sel_a"), (sel_b, "sel_b"),
    ]:
        nc.sync.dma_start(out=t_sb[:, :], in_=_rd(ins[name][:, :]))
    nc.sync.dma_start(out=hA[:, :], in_=_rd(ins["h0t"][:, :]))
    nc.sync.dma_start(out=chunk[0][:, :], in_=_rd(ins["dxc"][0, :, :]))
    if nchunk > 1:
        nc.sync.dma_start(out=chunk[1][:, :], in_=_rd(ins["dxc"][1, :, :]))
    nc.sync.dma_start(out=hB[H:H + 1, :], in_=_rd(ins["ones"][:, :]))
    nc.sync.dma_start(out=z1[HH:HH + 1, :], in_=_rd(ins["ones"][:, :]))
    nc.sync.dma_start(out=z2[HH:HH + 1, :], in_=_rd(ins["ones"][:, :]))
    nc.sync.dma_start(out=z3[HH:HH + 1, :], in_=_rd(ins["ones"][:, :]))

    # bcast tiles for t=0
    nc.tensor.matmul(pbc[:, 0:P], lhsT=_r(sel_a[:, :]), rhs=_r(chunk[0][:, 0:P]),
                     start=True, stop=True)
    nc.tensor.matmul(pbc[:, P:2 * P], lhsT=_r(sel_b[:, :]), rhs=_r(chunk[0][:, 0:P]),
                     start=True, stop=True)
    nc.vector.tensor_copy(bc[0][:, :], pbc[:, :])
    # substep-0 preactivation for t=0 (no red contributions yet)
    nc.tensor.matmul(pz1[0][:, :], lhsT=_r(w1[:, :]), rhs=_r(hA[:, :]),
                     start=True, stop=True)

    h_state = [hA, hB]                       # h_state[t % 2] holds state(t)
    uu2 = [uu_a, uu_b]                       # u of global substep g in uu2[g%2]
    C_SUB = [rw1_h, rw1_h, rw1_1]            # scale for h + c*k inputs
    W_RW1 = [rw1_6, rw1_3, rw1_3, rw1_6]     # RK4 combine weights into z1s0'
    W_RSEL = [rsel_6, rsel_3, rsel_3, rsel_6]

    def red_pair(dst, lhs, u, stop):
        nc.tensor.matmul(dst, lhsT=_r(lhs[:, :]), rhs=_r(u[:, 0:P]),
                         start=False, stop=False, skip_group_check=True)
        nc.tensor.matmul(dst, lhsT=_r(lhs[:, :]), rhs=_r(u[:, P:2 * P]),
                         start=False, stop=stop, skip_group_check=True)

    def base_mm(dst, lhs, rhs):
        nc.tensor.matmul(dst, lhsT=_r(lhs), rhs=_r(rhs),
                         start=True, stop=False, skip_group_check=True)

    for t in range(ts):
        last = t == ts - 1
        h_cur = h_state[t % 2]
        bct = bc[t % 2]
        # dx chunk prefetch (chunks 0,1 preloaded before the loop)
        ci = t // CH + 1
        if t % CH == 0 and 2 <= ci < nchunk:
            nc.sync.dma_start(out=chunk[ci % 2][:, :],
                              in_=_rd(ins["dxc"][ci, :, :]))

        for s in range(4):
            g = t * 4 + s
            u_prev = uu2[(g + 1) % 2]
            u_cur = uu2[g % 2]
            # relu of layer-1 preactivation (bias folded into the matmuls);
            # the reductions producing pz1[s] were emitted at the end of the
            # previous substep, ahead of everything below in the PE queue.
            nc.vector.tensor_scalar_max(z1[0:HH, :], pz1[s][:, :], 0.0)
            nc.tensor.matmul(pz23[:, 0:P], lhsT=_r(w2[:, :]), rhs=_r(z1[:, :]),
                             start=True, stop=True, skip_group_check=True)
            nc.vector.tensor_scalar_max(z2[0:HH, :], pz23[:, 0:P], 0.0)
            if s == 0 and t > 0:
                # materialize h(t); its producer (ph final red pair) sits just
                # ahead in the PE queue, so this lands early in the substep
                nc.vector.tensor_copy(h_cur[0:H, :], ph[:, :])
            nc.tensor.matmul(pz23[:, P:2 * P], lhsT=_r(w3[:, :]), rhs=_r(z2[:, :]),
                             start=True, stop=True, skip_group_check=True)
            nc.vector.tensor_scalar_max(z3[0:HH, :], pz23[:, P:2 * P], 0.0)
            nc.tensor.matmul(pf[:, 0:P], lhsT=_r(w4a[:, :]), rhs=_r(z3[:, :]),
                             start=True, stop=True, skip_group_check=True)
            nc.tensor.matmul(pf[:, P:2 * P], lhsT=_r(w4b[:, :]), rhs=_r(z3[:, :]),
                             start=True, stop=True, skip_group_check=True)
            # ---- PE fill work: executes during the tanh/mult gap ----
            if s == 0:
                base_mm(pz1[1][:, :], w1[:, :], h_cur[:, :])
                base_mm(ph[:, :], eye[:, :], h_cur[0:H, :])
                base_mm(pz1[2][:, :], w1[:, :], h_cur[:, :])
            elif s == 1:
                if not last:
                    base_mm(pz1[0][:, :], w1[:, :], h_cur[:, :])
                    red_pair(pz1[0][:, :], W_RW1[0], u_prev, False)
                red_pair(ph[:, :], W_RSEL[0], u_prev, False)
                base_mm(pz1[3][:, :], w1[:, :], h_cur[:, :])
            elif s == 2:
                if not last:
                    red_pair(pz1[0][:, :], W_RW1[1], u_prev, False)
                red_pair(ph[:, :], W_RSEL[1], u_prev, False)
                if not last:
                    tn = t + 1
                    sl = slice((tn % CH) * P, (tn % CH) * P + P)
                    cn = chunk[(tn // CH) % 2]
                    nc.tensor.matmul(pbc[:, 0:P], lhsT=_r(sel_a[:, :]),
                                     rhs=_r(cn[:, sl]), start=True, stop=True,
                                     skip_group_check=True)
                    nc.tensor.matmul(pbc[:, P:2 * P], lhsT=_r(sel_b[:, :]),
                                     rhs=_r(cn[:, sl]), start=True, stop=True,
                                     skip_group_check=True)
                    nc.vector.tensor_copy(bc[tn % 2][:, :], pbc[:, :])
            elif s == 3:
                if not last:
                    red_pair(pz1[0][:, :], W_RW1[2], u_prev, False)
                red_pair(ph[:, :], W_RSEL[2], u_prev, False)
            # ---- tail ----
            nc.scalar.activation(tt[:, :], pf[:, :], TANH)
            nc.vector.tensor_mul(u_cur[:, :], tt[:, :], bct[:, :])
            # on-path reductions of u_cur feeding the next substep
            if s < 3:
                red_pair(pz1[s + 1][:, :], C_SUB[s], u_cur, True)
            else:
                if not last:
                    red_pair(pz1[0][:, :], W_RW1[3], u_cur, True)
                red_pair(ph[:, :], W_RSEL[3], u_cur, True)

    h_fin = h_state[ts % 2]
    nc.vector.tensor_copy(h_fin[0:H, :], ph[:, :])
    src = h_fin[0:H, :].bitcast(F32) if USE_F32R else h_fin[0:H, :]
    nc.sync.dma_start(out=out_ap[:, :], in_=src)


_CACHE = {}


def _input_specs(ts):
    nchunk = (ts + CH - 1) // CH
    return {
        "w1": (H + 1, HH), "w2": (HH + 1, HH), "w3": (HH + 1, HH),
        "w4a": (HH + 1, 96), "w4b": (HH + 1, 96),
        "rw1_h": (96, HH), "rw1_1": (96, HH), "rw1_6": (96, HH),
        "rw1_3": (96, HH),
        "rsel_6": (96, H), "rsel_3": (96, H), "eye": (H, H),
        "sel_a": (D, 96), "sel_b": (D, 96),
        "h0t": (H + 1, P), "dxc": (nchunk, D, CH * P), "ones": (1, P),
    }


def build(ts=TS_FULL):
    if ts in _CACHE:
        return _CACHE[ts]
    nc = bacc.Bacc("TRN2", target_bir_lowering=False, debug=False,
                   enable_asserts=False, num_devices=NCORES)
    ins = {
        name: nc.dram_tensor(name, list(shape), F32, kind="ExternalInput").ap()
        for name, shape in _input_specs(ts).items()
    }
    out_ap = nc.dram_tensor("ht_out", [H, P], F32, kind="ExternalOutput").ap()
    with tile.TileContext(nc, trace_sim=False) as tc:
        with ExitStack() as ctx:
            _emit(ctx, tc, ins, out_ap, ts)
    nc.compile()
    _CACHE[ts] = nc
    return nc


def host_prep(coeffs, W0, b0, W1, b1, W2, b2, W3, b3, W4, b4, ts=TS_FULL):
    f32 = np.float32
    coeffs = np.ascontiguousarray(coeffs, dtype=f32)
    h0 = coeffs[:, 0, :] @ W0.astype(f32) + b0.astype(f32)      # [B, H]
    dX = coeffs[:, 1:ts + 1, :] - coeffs[:, :ts, :]             # [B, ts, D]

    W1 = W1.astype(f32)
    W4r = W4.astype(f32).reshape(HH, H, D)
    W4P = W4r.transpose(0, 2, 1).reshape(HH, D * H)             # cols d*32+i
    b4P = b4.astype(f32).reshape(H, D).T.reshape(D * H)
    RW1 = np.tile(W1, (3, 1)).astype(f32)                       # [96, HH]
    Rsel = np.tile(np.eye(H, dtype=f32), (3, 1))                # [96, H]
    sel_a = np.zeros((D, 96), f32)
    sel_b = np.zeros((D, 96), f32)
    for d in range(3):
        sel_a[d, 32 * d:32 * d + 32] = 1.0
        sel_b[d + 3, 32 * d:32 * d + 32] = 1.0

    shared = {
        "w1": np.concatenate([W1, b1.astype(f32)[None]], 0),
        "w2": np.concatenate([W2.astype(f32), b2.astype(f32)[None]], 0),
        "w3": np.concatenate([W3.astype(f32), b3.astype(f32)[None]], 0),
        "w4a": np.concatenate([W4P[:, :96], b4P[None, :96]], 0),
        "w4b": np.concatenate([W4P[:, 96:], b4P[None, 96:]], 0),
        "rw1_h": (0.5 * RW1), "rw1_1": RW1,
        "rw1_6": (RW1 / 6.0).astype(f32), "rw1_3": (RW1 / 3.0).astype(f32),
        "rsel_6": (Rsel / 6.0).astype(f32), "rsel_3": (Rsel / 3.0).astype(f32),
        "eye": np.eye(H, dtype=f32),
        "sel_a": sel_a, "sel_b": sel_b, "ones": np.ones((1, P), f32),
    }
    shared = {k: np.ascontiguousarray(v, dtype=f32) for k, v in shared.items()}

    nchunk = (ts + CH - 1) // CH
    in_maps = []
    for c in range(NCORES):
        sl = slice(c * P, (c + 1) * P)
        h0t = np.concatenate([h0[sl].T, np.ones((1, P), f32)], 0)
        dxt = dX[sl].transpose(1, 2, 0)                          # [ts, D, P]
        pad = np.zeros((nchunk * CH, D, P), f32)
        pad[:ts] = dxt
        dxc = pad.reshape(nchunk, CH, D, P).transpose(0, 2, 1, 3).reshape(
            nchunk, D, CH * P)
        m = dict(shared)
        m["h0t"] = np.ascontiguousarray(h0t, f32)
        m["dxc"] = np.ascontiguousarray(dxc, f32)
        in_maps.append(m)
    return in_maps


def run_device(in_maps, ts=TS_FULL, **kw):
    nc = build(ts)
    return run_bass_kernel_spmd(nc, in_maps, list(range(NCORES)), **kw)


def kernel(coeffs, W0, b0, W1, b1, W2, b2, W3, b3, W4, b4, Wf, bf):
    in_maps = host_prep(coeffs, W0, b0, W1, b1, W2, b2, W3, b3, W4, b4)
    res = run_device(in_maps)
    hT = np.stack([res.results[c]["ht_out"] for c in range(NCORES)])  # [8,H,P]
    h_all = hT.transpose(0, 2, 1).reshape(B, H)
    return (h_all @ Wf.astype(np.float32) + bf.astype(np.float32)).astype(
        np.float32)

